# revision 5
# baseline (speedup 1.0000x reference)
"""Trainium2 Bass kernel for EnhancedInvariantExtractor (v2).

Input  h [1_000_000, 120] f32:  per atom: 32 scalars | 16 vectors (l=1, dim 3)
                                | 8 tensors (l=2, dim 5).
Output [1_000_000, 204] f32: scalars(32) | vnorm(16) | tnorm(8) | vdots(120)
                             | tdots(28).

v2 strategy (8 cores, data-parallel over atoms; ~2e-2 rel-err budget):
- Device I/O in fp16: host converts h[:,32:120] -> hT [88, 125440] f16 per
  core (feature-major).  Scalars never touch the device (host pass-through).
- Chunks of 512 atoms; features on partitions.  Per chunk (ns @ roofline):
    PE   (2134): mm1 n2=S1'.sq | mm2 rexp=E4'.rinv | 4x mm3 u_k=P_k'.vu
                 | 4x mm4 d_k=(0.5 R_k)'.squ_k
    ACT  (~1930): ln(n2+eps) | exp(-L/2)=rinv | squares k0,k2 | copy d_a->f16
    DVE  (~2140): vu=x*rexp | squares k1,k3 | norms=n2*rinv (per group)
    Pool (~1920): sq=x*x | copy d_b->f16
- Pair-sum trick: cos(i,j) = |u_i+u_j|^2/2 - 1 with R_k carrying the 0.5;
  device ships d=|u|^2/2 in f16, host does the -1 and clip (free).
- 148 pairs in 4 chunks [42,42,40,24] (rows [126,126,128,120]); mm4 outputs
  packed two-per-PSUM-tile at partition offsets {0,64} (tile_position legal
  for <=64-row outputs), junk rows shipped and dropped by the host.
- Norm path per 4-chunk group on [128,512] strips: rinv=exp(-.5 ln(n2+eps^2)),
  norms = n2*rinv (f16 out).
"""

import sys

sys.path.insert(0, "/opt/trn_rl_repo")

import numpy as np

N_ATOMS = 1_000_000
N_CORES = 8
PER_CORE = N_ATOMS // N_CORES  # 125_000
CHUNK = 512
N_CHUNKS = 245
PADDED = CHUNK * N_CHUNKS  # 125_440
NF = 120
NOUT = 204
NV, NT = 16, 8
EPS2 = 1e-12
# pair chunking: k0 vec[0:42], k1 vec[42:84], k2 vec[84:120]+tens[0:4],
# k3 tens[4:28]
PAIR_CHUNK_BOUNDS = [0, 42, 84, 124, 148]
U_CHUNK_ROWS = [126, 126, 128, 120]
U_CHUNK_PAIRS = [42, 42, 40, 24]
# device dot-output layout: (tile, partition offset) per pair chunk
D_STRIPS = [("a", 0), ("a", 64), ("b", 0), ("b", 64)]
NOUT_DA = 128  # d_a tile rows (42 @ 0, 42 @ 64)
NOUT_DB = 88  # d_b tile rows (40 @ 0, 24 @ 64)

_CACHE = {}


def _vrow(i, d):
    return 3 * i + d


def _trow(t, d):
    return 48 + 5 * t + d


def _pair_list():
    pairs = []
    for i in range(NV):
        for j in range(i + 1, NV):
            pairs.append([(_vrow(i, d), _vrow(j, d)) for d in range(3)])
    for a in range(NT):
        for b in range(a + 1, NT):
            pairs.append([(_trow(a, d), _trow(b, d)) for d in range(5)])
    return pairs


def _stationaries():
    pairs = _pair_list()
    assert len(pairs) == 148

    s1 = np.zeros((88, 24), np.float16)
    for i in range(NV):
        for d in range(3):
            s1[_vrow(i, d), i] = 1.0
    for t in range(NT):
        for d in range(5):
            s1[_trow(t, d), 16 + t] = 1.0

    e4 = np.zeros((120, 88), np.float16)
    for j in range(4):
        e4[32 * j : 32 * j + 24, :] = s1.T

    p_ks, r_ks = [], []
    for k in range(4):
        lo, hi = PAIR_CHUNK_BOUNDS[k], PAIR_CHUNK_BOUNDS[k + 1]
        chunk_pairs = pairs[lo:hi]
        pk = len(chunk_pairs)
        rk = sum(len(c) for c in chunk_pairs)
        assert rk == U_CHUNK_ROWS[k] and pk == U_CHUNK_PAIRS[k]
        p_k = np.zeros((88, rk), np.float16)
        r_k = np.zeros((rk, pk), np.float16)
        r = 0
        for pl, comp in enumerate(chunk_pairs):
            for ri, rj in comp:
                p_k[ri, r] = 1.0
                p_k[rj, r] = 1.0
                r_k[r, pl] = 0.5
                r += 1
        p_ks.append(p_k)
        r_ks.append(r_k)
    return s1, e4, p_ks, r_ks


def _build_nc(n_chunks=N_CHUNKS, padded=None, reps=1):
    import concourse.bacc as bacc
    import concourse.bass as bass
    import concourse.tile as tile
    from concourse import mybir

    ACT = mybir.ActivationFunctionType
    f32, f16 = mybir.dt.float32, mybir.dt.float16
    if padded is None:
        padded = n_chunks * CHUNK

    import concourse.hw_specs as hw_specs

    if not getattr(hw_specs, "_invx_patched", False):
        _orig_tables = hw_specs.get_activation_tables

        def _only_nle(module_arch):
            tabs = _orig_tables(module_arch)
            keep = "natural_log_exp_and_others"
            assert keep in tabs
            return {
                name: (funcs if name == keep else set())
                for name, funcs in tabs.items()
            }

        hw_specs.get_activation_tables = _only_nle
        import concourse.bacc as _bacc_mod

        _bacc_mod.get_activation_tables = _only_nle
        hw_specs._invx_patched = True

    nc = bacc.Bacc("TRN2", target_bir_lowering=False, debug=False, num_devices=N_CORES)

    eps_t = nc.alloc_sbuf_tensor("const-f32-eps2", [128, 1], f32)
    nc.gpsimd.memset(eps_t.ap(), EPS2)
    nc.const_aps.aps[(f32, EPS2)] = eps_t.ap()
    nc.all_engine_barrier()

    ht_ext = nc.declare_dram_parameter("hT", [88, padded], f16, isOutput=False)
    s1_ext = nc.declare_dram_parameter("S1", [88, 24], f16, isOutput=False)
    e4_ext = nc.declare_dram_parameter("E4", [120, 88], f16, isOutput=False)
    p_exts = [
        nc.declare_dram_parameter(f"P{k}", [88, rk], f16, isOutput=False)
        for k, rk in enumerate(U_CHUNK_ROWS)
    ]
    r_exts = [
        nc.declare_dram_parameter(f"R{k}", [rk, pk], f16, isOutput=False)
        for k, (rk, pk) in enumerate(zip(U_CHUNK_ROWS, U_CHUNK_PAIRS))
    ]
    outd_ext = nc.declare_dram_parameter(
        "out_d", [NOUT_DA + NOUT_DB, padded], f16, isOutput=True
    )
    outn_ext = nc.declare_dram_parameter("out_n", [24, padded], f16, isOutput=True)

    with tile.TileContext(nc) as tc:
        with (
            tc.tile_pool(name="const", bufs=1) as cpool,
            tc.tile_pool(name="x", bufs=10) as xpool,
            tc.tile_pool(name="sq", bufs=4) as sqpool,
            tc.tile_pool(name="vu", bufs=4) as vupool,
            tc.tile_pool(name="squ", bufs=3) as squpool,
            tc.tile_pool(name="grp", bufs=2) as grppool,
            tc.tile_pool(name="oa", bufs=3) as oapool,
            tc.tile_pool(name="ob", bufs=3) as obpool,
            tc.tile_pool(name="ps_n2", bufs=1, space=bass.MemorySpace.PSUM) as ps_n2,
            tc.tile_pool(name="ps_re", bufs=1, space=bass.MemorySpace.PSUM) as ps_re,
            tc.tile_pool(name="ps_u01", bufs=1, space=bass.MemorySpace.PSUM) as ps_u01,
            tc.tile_pool(name="ps_u23", bufs=1, space=bass.MemorySpace.PSUM) as ps_u23,
            tc.tile_pool(name="ps_da", bufs=1, space=bass.MemorySpace.PSUM) as ps_da,
            tc.tile_pool(name="ps_db", bufs=1, space=bass.MemorySpace.PSUM) as ps_db,
        ):
            s1_t = cpool.tile([88, 24], f16)
            nc.sync.dma_start(out=s1_t[:], in_=s1_ext[:])
            e4_t = cpool.tile([120, 88], f16)
            nc.sync.dma_start(out=e4_t[:], in_=e4_ext[:])
            p_ts, r_ts = [], []
            for k, rk in enumerate(U_CHUNK_ROWS):
                p_t = cpool.tile([88, rk], f16, tag=f"P{k}")
                nc.sync.dma_start(out=p_t[:], in_=p_exts[k][:])
                p_ts.append(p_t)
                r_t = cpool.tile([rk, U_CHUNK_PAIRS[k]], f16, tag=f"R{k}")
                nc.sync.dma_start(out=r_t[:], in_=r_exts[k][:])
                r_ts.append(r_t)

            def phase_C(c, state):
                j = c % 4
                rexp = ps_re.tile([88, CHUNK], f32, tag="rexp")
                nc.tensor.matmul(
                    rexp[:],
                    e4_t[32 * j : 32 * j + 24, :],
                    state["rinvg"][32 * j : 32 * j + 24, :],
                    tile_position=(32 * j, 0),
                )
                vu_t = vupool.tile([88, CHUNK], f16, tag="vu")
                nc.vector.tensor_mul(vu_t[:], state["xs"][c][:], rexp[:])
                state["vus"][c] = vu_t

            def phase_D(c, state):
                cols = slice(c * CHUNK, (c + 1) * CHUNK)
                # u tiles pair two k-chunks side by side in the free dim so
                # one ACT Square covers both (amortizes the PSUM access
                # penalty and halves the ACT op count)
                u01 = ps_u01.tile([126, 2 * CHUNK], f32, tag="u01")
                nc.tensor.matmul(u01[:, 0:CHUNK], p_ts[0][:], state["vus"][c][:])
                nc.tensor.matmul(
                    u01[:, CHUNK : 2 * CHUNK], p_ts[1][:], state["vus"][c][:]
                )
                squ01 = squpool.tile([126, 2 * CHUNK], f16, tag="squ01")
                nc.scalar.activation(squ01[:], u01[:], ACT.Square, bias=0.0, scale=1.0)
                u23 = ps_u23.tile([128, 2 * CHUNK], f32, tag="u23")
                nc.tensor.matmul(u23[:, 0:CHUNK], p_ts[2][:], state["vus"][c][:])
                nc.tensor.matmul(
                    u23[0:120, CHUNK : 2 * CHUNK], p_ts[3][:], state["vus"][c][:]
                )
                squ23 = squpool.tile([128, 2 * CHUNK], f16, tag="squ23")
                nc.scalar.activation(squ23[:], u23[:], ACT.Square, bias=0.0, scale=1.0)
                da = ps_da.tile([NOUT_DA, CHUNK], f32, tag="da")
                db = ps_db.tile([NOUT_DB, CHUNK], f32, tag="db")
                nc.tensor.matmul(
                    da[0:42, :], r_ts[0][:], squ01[:, 0:CHUNK], tile_position=(0, 0)
                )
                nc.tensor.matmul(
                    da[64:106, :],
                    r_ts[1][:],
                    squ01[:, CHUNK : 2 * CHUNK],
                    tile_position=(0, 64),
                )
                nc.tensor.matmul(
                    db[0:40, :], r_ts[2][:], squ23[:, 0:CHUNK], tile_position=(0, 0)
                )
                nc.tensor.matmul(
                    db[64:88, :],
                    r_ts[3][:],
                    squ23[0:120, CHUNK : 2 * CHUNK],
                    tile_position=(0, 64),
                )
                oa = oapool.tile([NOUT_DA, CHUNK], f16, tag="oa")
                nc.vector.tensor_copy(oa[:], da[:])
                ob = obpool.tile([NOUT_DB, CHUNK], f16, tag="ob")
                nc.vector.tensor_copy(ob[:], db[:])
                nc.sync.dma_start(out=outd_ext[0:NOUT_DA, cols], in_=oa[:])
                nc.sync.dma_start(
                    out=outd_ext[NOUT_DA : NOUT_DA + NOUT_DB, cols], in_=ob[:]
                )

            for rep in range(reps):
                for g in range(0, n_chunks, 4):
                    chunks = list(range(g, min(g + 4, n_chunks)))

                    # phase A: load, square, per-chunk n2 into strip j
                    n2g = ps_n2.tile([128, CHUNK], f32, tag="n2g")
                    state = {"xs": {}, "vus": {}}
                    for c in chunks:
                        j = c % 4
                        x_t = xpool.tile([88, CHUNK], f16, tag="x")
                        nc.sync.dma_start(
                            out=x_t[:], in_=ht_ext[:, c * CHUNK : (c + 1) * CHUNK]
                        )
                        state["xs"][c] = x_t
                        sq_t = sqpool.tile([88, CHUNK], f16, tag="sq")
                        nc.gpsimd.tensor_mul(sq_t[:], x_t[:], x_t[:])
                        nc.tensor.matmul(
                            n2g[32 * j : 32 * j + 24, :],
                            s1_t[:],
                            sq_t[:],
                            tile_position=(0, 32 * j),
                        )

                    # phase B: group norm path
                    lng = grppool.tile([128, CHUNK], f32, tag="lng")
                    nc.scalar.activation(lng[:], n2g[:], ACT.Ln, bias=EPS2, scale=1.0)
                    rinvg = grppool.tile([128, CHUNK], f16, tag="rinvg")
                    nc.scalar.activation(rinvg[:], lng[:], ACT.Exp, bias=0.0, scale=-0.5)
                    state["rinvg"] = rinvg
                    normn = grppool.tile([128, CHUNK], f16, tag="normn")
                    nc.vector.tensor_mul(normn[:], n2g[:], rinvg[:])
                    for c in chunks:
                        j = c % 4
                        nc.sync.dma_start(
                            out=outn_ext[:, c * CHUNK : (c + 1) * CHUNK],
                            in_=normn[32 * j : 32 * j + 24, :],
                        )

                    # phases C/D interleaved: C0 C1 D0 C2 D1 C3 D2 D3 keeps
                    # the PE fed while vu (DVE) completes
                    for idx, c in enumerate(chunks):
                        phase_C(c, state)
                        if idx >= 1:
                            phase_D(chunks[idx - 1], state)
                    phase_D(chunks[-1], state)

    nc.compile()
    return nc


def _get_nc():
    if "nc" not in _CACHE:
        _CACHE["nc"] = _build_nc()
    return _CACHE["nc"]


def _make_in_map(shard, stat):
    """shard [n<=PADDED, 120] f32 -> hT [88, PADDED] f16 (feature-major)."""
    buf = np.ones((PADDED, 88), np.float16)
    buf[: shard.shape[0]] = shard[:, 32:120]
    return {"hT": np.ascontiguousarray(buf.T), **stat}


def _stat_map():
    s1, e4, p_ks, r_ks = _stationaries()
    stat = {"S1": s1, "E4": e4}
    for k in range(4):
        stat[f"P{k}"] = p_ks[k]
        stat[f"R{k}"] = r_ks[k]
    return stat


def _assemble(dev_d, dev_n, h_shard, n):
    """device outputs + host scalars -> [n, 204] reference layout."""
    o = np.empty((n, NOUT), np.float32)
    o[:, 0:32] = h_shard[:n, 0:32]
    o[:, 32:48] = dev_n[0:16, :n].T
    o[:, 48:56] = dev_n[16:24, :n].T
    d = np.concatenate(
        [dev_d[0:42, :n], dev_d[64:106, :n], dev_d[128:168, :n], dev_d[192:216, :n]],
        axis=0,
    )
    dots = np.clip(d.astype(np.float32).T - 1.0, -1.0, 1.0)
    o[:, 56:176] = dots[:, 0:120]
    o[:, 176:204] = dots[:, 120:148]
    return o


def _run_pjrt(nc, in_maps):
    """Execute the Bass module on N_CORES devices via PJRT/shard_map with
    per-device buffer assembly and per-shard fetch (avoids giant host
    concats, which trip transfer limits on the axon path)."""
    import jax
    from jax.sharding import Mesh, NamedSharding, PartitionSpec
    from jax.experimental.shard_map import shard_map
    from concourse import mybir
    from concourse.bass2jax import (
        _bass_exec_p,
        install_neuronx_cc_hook,
        partition_id_tensor,
    )

    install_neuronx_cc_hook()
    partition_name = nc.partition_id_tensor.name if nc.partition_id_tensor else None
    in_names, out_names, out_avals = [], [], []
    for alloc in nc.m.functions[0].allocations:
        if not isinstance(alloc, mybir.MemoryLocationSet):
            continue
        name = alloc.memorylocations[0].name
        if alloc.kind == "ExternalInput":
            if name != partition_name:
                in_names.append(name)
        elif alloc.kind == "ExternalOutput":
            out_names.append(name)
            shape = tuple(alloc.tensor_shape)
            dtype = mybir.dt.np(alloc.dtype)
            out_avals.append(jax.core.ShapedArray(shape, dtype))
    n_params = len(in_names)
    n_outs = len(out_avals)
    all_in_names = list(in_names) + out_names
    if partition_name is not None:
        all_in_names.append(partition_name)
    donate = tuple(range(n_params, n_params + n_outs))

    def _body(*args):
        operands = list(args)
        if partition_name is not None:
            operands.append(partition_id_tensor())
        outs = _bass_exec_p.bind(
            *operands,
            out_avals=tuple(out_avals),
            in_names=tuple(all_in_names),
            out_names=tuple(out_names),
            lowering_input_output_aliases=(),
            sim_require_finite=True,
            sim_require_nnan=True,
            nc=nc,
        )
        return tuple(outs)

    devices = jax.devices()[:N_CORES]
    mesh = Mesh(np.asarray(devices), ("core",))
    sharding = NamedSharding(mesh, PartitionSpec("core"))
    fn = jax.jit(
        shard_map(
            _body,
            mesh=mesh,
            in_specs=(PartitionSpec("core"),) * (n_params + n_outs),
            out_specs=(PartitionSpec("core"),) * n_outs,
            check_rep=False,
        ),
        donate_argnums=donate,
        keep_unused=True,
    )

    def make_global(per_core_arrays):
        a0 = per_core_arrays[0]
        gshape = (N_CORES * a0.shape[0],) + a0.shape[1:]
        bufs = [
            jax.device_put(per_core_arrays[c], devices[c]) for c in range(N_CORES)
        ]
        return jax.make_array_from_single_device_arrays(gshape, sharding, bufs)

    g_ins = [
        make_global([np.asarray(in_maps[c][nm]) for c in range(N_CORES)])
        for nm in in_names
    ]
    g_zeros = [
        make_global([np.zeros(av.shape, av.dtype) for _ in range(N_CORES)])
        for av in out_avals
    ]
    outs = fn(*g_ins, *g_zeros)
    jax.block_until_ready(outs)

    results = [dict() for _ in range(N_CORES)]
    for i, nm in enumerate(out_names):
        shards = sorted(
            outs[i].addressable_shards, key=lambda s: devices.index(s.device)
        )
        for c, sh in enumerate(shards):
            results[c][nm] = np.asarray(sh.data)
    return results


def kernel(h):
    h = np.asarray(h, dtype=np.float32)
    assert h.shape == (N_ATOMS, NF)

    nc = _get_nc()
    stat = _stat_map()
    in_maps = [
        _make_in_map(h[c * PER_CORE : (c + 1) * PER_CORE], stat)
        for c in range(N_CORES)
    ]
    res = _run_pjrt(nc, in_maps)

    out = np.empty((N_ATOMS, NOUT), np.float32)
    for c in range(N_CORES):
        out[c * PER_CORE : (c + 1) * PER_CORE] = _assemble(
            res[c]["out_d"],
            res[c]["out_n"],
            h[c * PER_CORE : (c + 1) * PER_CORE],
            PER_CORE,
        )
    return out
